# revision 3
# baseline (speedup 1.0000x reference)
"""CLIP encoder layer on 8 Trainium2 NeuronCores, data-parallel over batch.

Full (unsharded) inputs -> full output.  Each core runs the whole layer for
one batch element (B == 8 == n_cores), so there are no collectives.

Layout strategy per core:
  - LayerNorm in token-major layout (bn_stats over free dim), then the
    normalized tensor is transposed per 128x128 block on the PE; the LN
    affine (scale/bias, per feature == per partition after transpose) is
    fused into the PSUM eviction.
  - Q/K are produced feature-major [D, L]; V is produced feature-major then
    transposed into token-major V65 [L, 16*65] with a ones column per head
    (fused softmax denominator).
  - Scores are computed key-major S^T[keys, q] so softmax reduces over the
    PSUM partition dim via the matmul itself: exp is a single ACT op with
    the attention scale and the additive key mask folded into scale/bias,
    and O^T = V65^T @ expS accumulates both numerator and denominator.
  - The per-head division defers to a PE ones-broadcast of 1/denom plus one
    DVE multiply on [64, L].
  - MLP runs in two token halves so the gelu activations fit in SBUF.
  - All big matmuls run in float32r (full PE rate at N>=256, ~1e-4 rounding).

Weights are re-tiled on the host so every weight DMA is a contiguous
[128, 8, 128] (512 KB) transfer.
"""
import numpy as np

import concourse.bacc as bacc
import concourse.tile as tile
from concourse import mybir
from concourse.masks import make_identity

B, L, D = 8, 1024, 1024
H, HD, FF = 16, 64, 4096
EPS = 1e-5
P = 128
NCORES = 8
TC = L // P      # 8 token tiles
FC = D // P      # 8 feature tiles
MC = FF // P     # 32 ff tiles

f32 = mybir.dt.float32
f32r = mybir.dt.float32r
i32 = mybir.dt.int32
AF = mybir.ActivationFunctionType
ALU = mybir.AluOpType


def build_nc():
    nc = bacc.Bacc(None)

    # ---- DRAM I/O ----
    xd = nc.dram_tensor("x", [L, D], f32, kind="ExternalInput")
    maskd = nc.dram_tensor("mask", [L], i32, kind="ExternalInput")
    wqr = nc.dram_tensor("wqr", [FC, P, FC, P], f32r, kind="ExternalInput")
    wkr = nc.dram_tensor("wkr", [FC, P, FC, P], f32r, kind="ExternalInput")
    wvr = nc.dram_tensor("wvr", [FC, P, FC, P], f32r, kind="ExternalInput")
    wor = nc.dram_tensor("wor", [FC, P, FC, P], f32r, kind="ExternalInput")
    w1r = nc.dram_tensor("w1r", [MC, P, FC, P], f32r, kind="ExternalInput")
    w2r = nc.dram_tensor("w2r", [FC, P, MC, P], f32r, kind="ExternalInput")
    bqd = nc.dram_tensor("bq", [D], f32, kind="ExternalInput")
    bkd = nc.dram_tensor("bk", [D], f32, kind="ExternalInput")
    bvd = nc.dram_tensor("bv", [D], f32, kind="ExternalInput")
    bod = nc.dram_tensor("bo", [D], f32, kind="ExternalInput")
    b1d = nc.dram_tensor("b1", [FF], f32, kind="ExternalInput")
    b2d = nc.dram_tensor("b2", [D], f32, kind="ExternalInput")
    ln1sd = nc.dram_tensor("ln1_s", [D], f32, kind="ExternalInput")
    ln1bd = nc.dram_tensor("ln1_b", [D], f32, kind="ExternalInput")
    ln2sd = nc.dram_tensor("ln2_s", [D], f32, kind="ExternalInput")
    ln2bd = nc.dram_tensor("ln2_b", [D], f32, kind="ExternalInput")
    yd = nc.dram_tensor("y", [L, D], f32, kind="ExternalOutput")
    x1d = nc.dram_tensor("x1_scratch", [L, D], f32)

    with tile.TileContext(nc) as tc:
        _emit(nc, tc, locals())
    nc.compile()
    return nc


def _emit(nc, tc, d):
    from contextlib import ExitStack

    xd, maskd, yd, x1d = d["xd"], d["maskd"], d["yd"], d["x1d"]
    with ExitStack() as ctx:
        big = ctx.enter_context(tc.tile_pool(name="big", bufs=3))
        v65p = ctx.enter_context(tc.tile_pool(name="v65p", bufs=1))
        expp = ctx.enter_context(tc.tile_pool(name="expp", bufs=2))
        nst = ctx.enter_context(tc.tile_pool(name="nst", bufs=4))
        wp = ctx.enter_context(tc.tile_pool(name="wp", bufs=4))
        yp = ctx.enter_context(tc.tile_pool(name="yp", bufs=4))
        otp = ctx.enter_context(tc.tile_pool(name="otp", bufs=2))
        bcp = ctx.enter_context(tc.tile_pool(name="bcp", bufs=2))
        smal = ctx.enter_context(tc.tile_pool(name="smal", bufs=1))
        mm = ctx.enter_context(tc.tile_pool(name="mm", bufs=6, space="PSUM"))
        tpp = ctx.enter_context(tc.tile_pool(name="tpp", bufs=2, space="PSUM"))

        # ---------------- constants ----------------
        ident_st = smal.tile([P, P], f32, tag="ident_st")
        make_identity(nc, ident_st[:])
        ident = smal.tile([P, P], f32r, tag="ident")
        nc.vector.tensor_copy(out=ident[:], in_=ident_st[:].bitcast(f32r))

        ones_st = smal.tile([1, 64], f32, tag="ones_st")
        nc.vector.memset(ones_st[:], 1.0)
        ones_r = smal.tile([1, 64], f32r, tag="ones_r")
        nc.vector.tensor_copy(out=ones_r[:], in_=ones_st[:].bitcast(f32r))

        def load_vec(name, dram, n):
            t = smal.tile([P, n // P], f32, tag=name)
            nc.gpsimd.dma_start(out=t[:], in_=dram[:].rearrange("(c p) -> p c", p=P))
            return t

        bqt = load_vec("bqt", d["bqd"], D)
        bkt = load_vec("bkt", d["bkd"], D)
        bvt = load_vec("bvt", d["bvd"], D)
        bot = load_vec("bot", d["bod"], D)
        b1t = load_vec("b1t", d["b1d"], FF)
        b2t = load_vec("b2t", d["b2d"], D)
        l1s = load_vec("l1s", d["ln1sd"], D)
        l1b = load_vec("l1b", d["ln1bd"], D)
        l2s = load_vec("l2s", d["ln2sd"], D)
        l2b = load_vec("l2b", d["ln2bd"], D)

        epst = smal.tile([P, 1], f32, tag="epst")
        nc.vector.memset(epst[:], EPS)

        # additive key mask: (m - 1) * 1e30  ->  0 or -1e30
        mi = smal.tile([P, TC], i32, tag="mi")
        nc.gpsimd.dma_start(out=mi[:], in_=maskd[:].rearrange("(t p) -> p t", p=P))
        mf = smal.tile([P, TC], f32, tag="mf")
        nc.vector.tensor_copy(out=mf[:], in_=mi[:])
        fmask = smal.tile([P, TC], f32, tag="fmask")
        nc.vector.tensor_scalar(out=fmask[:], in0=mf[:], scalar1=1.0, scalar2=1e30,
                                op0=ALU.subtract, op1=ALU.mult)

        # V65: token-major V with a ones column per head
        v65 = v65p.tile([P, TC, H * 65], f32r, tag="v65")
        ones_col_st = smal.tile([P, TC, H], f32, tag="ones_col_st")
        nc.vector.memset(ones_col_st[:], 1.0)
        v65_ones = v65[:, :, :].rearrange("p t (h c) -> p t h c", c=65)[:, :, :, 64]
        nc.vector.tensor_copy(out=v65_ones, in_=ones_col_st[:].bitcast(f32r))

        # ---------------- helpers ----------------
        def layernorm_tile(x_tc, sc_pool):
            """token-major [128, D] -> normalized f32r tile (no affine)."""
            st = sc_pool.tile([P, 2, nc.vector.BN_STATS_DIM], f32, tag="bnst")
            xg = x_tc[:].rearrange("p (s f) -> p s f", s=2)
            for s in range(2):
                nc.vector.bn_stats(out=st[:, s, :], in_=xg[:, s, :])
            mv = sc_pool.tile([P, nc.vector.BN_AGGR_DIM], f32, tag="bnmv")
            nc.vector.bn_aggr(out=mv[:], in_=st[:])
            sd = sc_pool.tile([P, 1], f32, tag="bnsd")
            nc.scalar.activation(sd[:], mv[:, 1:2], AF.Sqrt, bias=epst[:], scale=1.0)
            r0 = sc_pool.tile([P, 1], f32, tag="bnr0")
            nc.vector.reciprocal(out=r0[:], in_=sd[:])
            # one Newton step: r1 = r0 * (2 - sd*r0)
            t1 = sc_pool.tile([P, 1], f32, tag="bnt1")
            nc.vector.tensor_mul(t1[:], sd[:], r0[:])
            nc.vector.tensor_scalar(out=t1[:], in0=t1[:], scalar1=-1.0, scalar2=2.0,
                                    op0=ALU.mult, op1=ALU.add)
            rstd = sc_pool.tile([P, 1], f32, tag="bnrstd")
            nc.vector.tensor_mul(rstd[:], r0[:], t1[:])
            n_tc = nst.tile([P, D], f32r, tag="nstage")
            nc.vector.tensor_scalar(out=n_tc[:], in0=x_tc[:], scalar1=mv[:, 0:1],
                                    scalar2=rstd[:], op0=ALU.subtract, op1=ALU.mult)
            return n_tc

        # ---------------- LN1 -> h1T (feature-major, f32r) ----------------
        h1T = big.tile([P, FC, L], f32r, tag="big")
        for t in range(TC):
            x_tc = nst.tile([P, D], f32, tag="nstage")
            nc.sync.dma_start(out=x_tc[:], in_=xd[t * P:(t + 1) * P, :])
            n_tc = layernorm_tile(x_tc, smal)
            for c in range(FC):
                tp = tpp.tile([P, P], f32r, tag="tp")
                nc.tensor.transpose(tp[:], n_tc[:, c * P:(c + 1) * P], ident[:])
                nc.scalar.activation(h1T[:, c, t * P:(t + 1) * P], tp[:].bitcast(f32),
                                     AF.Identity, bias=l1b[:, c:c + 1],
                                     scale=l1s[:, c:c + 1])

        # ---------------- Q/K projections (feature-major) ----------------
        qT = big.tile([P, FC, L], f32r, tag="big")
        kT = big.tile([P, FC, L], f32r, tag="big")
        for dst, wsrc, bias in ((qT, d["wqr"], bqt), (kT, d["wkr"], bkt)):
            for fc in range(FC):
                wt = wp.tile([P, FC, P], f32r, tag="w")
                nc.sync.dma_start(out=wt[:], in_=wsrc[fc])
                for half in range(2):
                    ps = mm.tile([P, 512], f32, tag="mm")
                    for kt in range(FC):
                        nc.tensor.matmul(
                            ps[:], wt[:, kt, :],
                            h1T[:, kt, half * 512:(half + 1) * 512],
                            start=(kt == 0), stop=(kt == FC - 1))
                    nc.vector.tensor_scalar_add(
                        out=dst[:, fc, half * 512:(half + 1) * 512],
                        in0=ps[:].bitcast(f32r), scalar1=bias[:, fc:fc + 1])

        # ---------------- V projection -> V65 (token-major + ones) -------
        for fc in range(FC):
            wt = wp.tile([P, FC, P], f32r, tag="w")
            nc.sync.dma_start(out=wt[:], in_=d["wvr"][fc])
            vt_fc = nst.tile([P, L], f32r, tag="nstage")
            for half in range(2):
                ps = mm.tile([P, 512], f32, tag="mm")
                for kt in range(FC):
                    nc.tensor.matmul(
                        ps[:], wt[:, kt, :],
                        h1T[:, kt, half * 512:(half + 1) * 512],
                        start=(kt == 0), stop=(kt == FC - 1))
                nc.vector.tensor_scalar_add(
                    out=vt_fc[:, half * 512:(half + 1) * 512],
                    in0=ps[:].bitcast(f32r), scalar1=bvt[:, fc:fc + 1])
            # transpose [vfeat, tok] blocks into token-major V65 homes
            for t in range(TC):
                tp = tpp.tile([P, P], f32r, tag="tp")
                nc.tensor.transpose(tp[:], vt_fc[:, t * P:(t + 1) * P], ident[:])
                dst = v65[:, t, :].rearrange("p (h c) -> p h c", c=65)[:, 2 * fc:2 * fc + 2, :64]
                nc.vector.tensor_copy(out=dst, in_=tp[:].rearrange("p (h c) -> p h c", c=64))

        # ---------------- attention ----------------
        attnT = big.tile([P, FC, L], f32r, tag="big")
        for h in range(H):
            p0 = (h % 2) * 64
            hc = h // 2
            ot_ps = [mm.tile([65, 512], f32, tag="mm", name=f"ot_ps{h}_{i}") for i in range(2)]
            for kt in range(TC):
                st_ps = [mm.tile([P, 512], f32, tag="mm", name=f"st_ps{h}_{kt}_{i}") for i in range(2)]
                for half in range(2):
                    nc.tensor.matmul(
                        st_ps[half][:],
                        kT[p0:p0 + 64, hc, kt * P:(kt + 1) * P],
                        qT[p0:p0 + 64, hc, half * 512:(half + 1) * 512],
                        start=True, stop=True)
                es = expp.tile([P, L], f32r, tag="expS")
                for half in range(2):
                    nc.scalar.activation(
                        es[:, half * 512:(half + 1) * 512], st_ps[half][:],
                        AF.Exp, bias=fmask[:, kt:kt + 1], scale=0.125)
                for half in range(2):
                    nc.tensor.matmul(
                        ot_ps[half][:],
                        v65[:, kt, h * 65:(h + 1) * 65],
                        es[:, half * 512:(half + 1) * 512],
                        start=(kt == 0), stop=(kt == TC - 1))
            # epilogue: 1/denom, PE broadcast, divide
            for half in range(2):
                r0 = smal.tile([1, 512], f32, tag="hr0")
                nc.vector.reciprocal(out=r0[:], in_=ot_ps[half][64:65, :])
                t1 = smal.tile([1, 512], f32, tag="ht1")
                nc.vector.tensor_mul(t1[:], ot_ps[half][64:65, :], r0[:])
                nc.vector.tensor_scalar(out=t1[:], in0=t1[:], scalar1=-1.0,
                                        scalar2=2.0, op0=ALU.mult, op1=ALU.add)
                rec = smal.tile([1, 512], f32r, tag="hrec")
                nc.vector.tensor_mul(rec[:], r0[:], t1[:])
                bc_ps = tpp.tile([64, 512], f32, tag="tp")
                nc.tensor.matmul(bc_ps[:], ones_r[:], rec[:], start=True, stop=True)
                bc = bcp.tile([64, 512], f32, tag="bc")
                nc.scalar.activation(bc[:], bc_ps[:], AF.Copy)
                if p0 == 0:
                    nc.vector.tensor_mul(
                        attnT[0:64, hc, half * 512:(half + 1) * 512],
                        ot_ps[half][0:64, :].bitcast(f32r), bc[:].bitcast(f32r))
                else:
                    od = otp.tile([64, 512], f32r, tag="otdiv")
                    nc.vector.tensor_mul(od[:], ot_ps[half][0:64, :].bitcast(f32r),
                                         bc[:].bitcast(f32r))
                    nc.gpsimd.dma_start(
                        out=attnT[p0:p0 + 64, hc, half * 512:(half + 1) * 512],
                        in_=od[:])

        # ---------------- out projection -> ZT (feature-major) ----------
        zT = big.tile([P, FC, L], f32r, tag="big")
        for m in range(FC):
            wt = wp.tile([P, FC, P], f32r, tag="w")
            nc.sync.dma_start(out=wt[:], in_=d["wor"][m])
            for half in range(2):
                ps = mm.tile([P, 512], f32, tag="mm")
                for ks in range(FC):
                    nc.tensor.matmul(
                        ps[:], wt[:, ks, :],
                        attnT[:, ks, half * 512:(half + 1) * 512],
                        start=(ks == 0), stop=(ks == FC - 1))
                nc.vector.tensor_scalar_add(
                    out=zT[:, m, half * 512:(half + 1) * 512],
                    in0=ps[:].bitcast(f32r), scalar1=bot[:, m:m + 1])

        # -------- residual 1 + LN2 -> h2T, x1 -> DRAM scratch ------------
        h2T = big.tile([P, FC, L], f32r, tag="big")
        for t in range(TC):
            x1_tc = nst.tile([P, D], f32, tag="nstage")
            nc.sync.dma_start(out=x1_tc[:], in_=xd[t * P:(t + 1) * P, :])
            for c in range(FC):
                tp = tpp.tile([P, P], f32r, tag="tp")
                nc.tensor.transpose(tp[:], zT[:, c, t * P:(t + 1) * P], ident[:])
                nc.vector.tensor_add(x1_tc[:, c * P:(c + 1) * P],
                                     x1_tc[:, c * P:(c + 1) * P],
                                     tp[:].bitcast(f32))
            nc.sync.dma_start(out=x1d[t * P:(t + 1) * P, :], in_=x1_tc[:])
            n_tc = layernorm_tile(x1_tc, smal)
            for c in range(FC):
                tp = tpp.tile([P, P], f32r, tag="tp")
                nc.tensor.transpose(tp[:], n_tc[:, c * P:(c + 1) * P], ident[:])
                nc.scalar.activation(h2T[:, c, t * P:(t + 1) * P], tp[:].bitcast(f32),
                                     AF.Identity, bias=l2b[:, c:c + 1],
                                     scale=l2s[:, c:c + 1])

        # ---------------- MLP in two token halves ----------------
        for half in range(2):
            g0 = big.tile([P, 16, 512], f32r, tag="big")
            g1 = big.tile([P, 16, 512], f32r, tag="big")
            gs = (g0, g1)
            for m in range(MC):
                wt = wp.tile([P, FC, P], f32r, tag="w")
                nc.sync.dma_start(out=wt[:], in_=d["w1r"][m])
                ps = mm.tile([P, 512], f32, tag="mm")
                for kt in range(FC):
                    nc.tensor.matmul(
                        ps[:], wt[:, kt, :],
                        h2T[:, kt, half * 512:(half + 1) * 512],
                        start=(kt == 0), stop=(kt == FC - 1))
                nc.scalar.activation(gs[m // 16][:, m % 16, :], ps[:],
                                     AF.Gelu_apprx_sigmoid,
                                     bias=b1t[:, m:m + 1], scale=1.0)
            # y tiles for this half, initialized from x1
            y_tcs = []
            for tq in range(4):
                t = half * 4 + tq
                y_tc = yp.tile([P, D], f32, tag="y")
                nc.sync.dma_start(out=y_tc[:], in_=x1d[t * P:(t + 1) * P, :])
                y_tcs.append(y_tc)
            for m in range(FC):
                for g in range(4):
                    wt = wp.tile([P, FC, P], f32r, tag="w")
                    nc.sync.dma_start(out=wt[:], in_=d["w2r"][m, :, g * 8:(g + 1) * 8, :])
                    ps = mm.tile([P, 512], f32, tag="mm", name=f"fc2ps{half}_{m}") if g == 0 else ps
                    for kt in range(FC):
                        ktg = g * 8 + kt
                        nc.tensor.matmul(
                            ps[:], wt[:, kt, :],
                            gs[ktg // 16][:, ktg % 16, :],
                            start=(ktg == 0), stop=(ktg == MC - 1))
                mt = nst.tile([P, 512], f32, tag="nstage")
                nc.vector.tensor_scalar_add(out=mt[:], in0=ps[:],
                                            scalar1=b2t[:, m:m + 1])
                mtr = nst.tile([P, 512], f32r, tag="nstage")
                nc.vector.tensor_copy(out=mtr[:], in_=mt[:].bitcast(f32r))
                for tq in range(4):
                    tp = tpp.tile([P, P], f32r, tag="tp")
                    nc.tensor.transpose(tp[:], mtr[:, tq * P:(tq + 1) * P], ident[:])
                    nc.vector.tensor_add(y_tcs[tq][:, m * P:(m + 1) * P],
                                         y_tcs[tq][:, m * P:(m + 1) * P],
                                         tp[:].bitcast(f32))
            for tq in range(4):
                t = half * 4 + tq
                nc.sync.dma_start(out=yd[t * P:(t + 1) * P, :], in_=y_tcs[tq][:])


_NC_CACHE = {}


def _get_nc():
    if "nc" not in _NC_CACHE:
        _NC_CACHE["nc"] = build_nc()
    return _NC_CACHE["nc"]


def _retile(w, kslices, mslices):
    """[K, M] -> [mslices, 128, kslices, 128] with blk[m,p,k,c] = w[k*128+p, m*128+c]."""
    K, M = w.shape
    assert K == kslices * P and M == mslices * P
    return np.ascontiguousarray(
        w.reshape(kslices, P, mslices, P).transpose(2, 1, 0, 3))


def make_in_maps(x, attention_mask, wq, bq, wk, bk, wv, bv, wo, bo,
                 ln1_s, ln1_b, ln2_s, ln2_b, w1, b1, w2, b2):
    f = np.asarray
    shared = {
        "wqr": _retile(f(wq, dtype=np.float32), FC, FC),
        "wkr": _retile(f(wk, dtype=np.float32), FC, FC),
        "wvr": _retile(f(wv, dtype=np.float32), FC, FC),
        "wor": _retile(f(wo, dtype=np.float32), FC, FC),
        "w1r": _retile(f(w1, dtype=np.float32), FC, MC),
        "w2r": _retile(f(w2, dtype=np.float32), MC, FC),
        "bq": f(bq, dtype=np.float32), "bk": f(bk, dtype=np.float32),
        "bv": f(bv, dtype=np.float32), "bo": f(bo, dtype=np.float32),
        "b1": f(b1, dtype=np.float32), "b2": f(b2, dtype=np.float32),
        "ln1_s": f(ln1_s, dtype=np.float32), "ln1_b": f(ln1_b, dtype=np.float32),
        "ln2_s": f(ln2_s, dtype=np.float32), "ln2_b": f(ln2_b, dtype=np.float32),
    }
    x = f(x, dtype=np.float32)
    m = f(attention_mask, dtype=np.int32)
    return [dict(shared, x=np.ascontiguousarray(x[c]),
                 mask=np.ascontiguousarray(m[c])) for c in range(NCORES)]


def kernel(**inputs):
    from concourse.bass_utils import run_bass_kernel_spmd

    nc = _get_nc()
    in_maps = make_in_maps(**inputs)
    res = run_bass_kernel_spmd(nc, in_maps, core_ids=list(range(NCORES)))
    out = np.stack([res.results[c]["y"] for c in range(NCORES)], axis=0)
    return out.astype(np.float32)


# revision 4
# speedup vs baseline: 8.7910x; 8.7910x over previous
"""CLIP encoder layer on 8 Trainium2 NeuronCores, data-parallel over batch.

Full (unsharded) inputs -> full output.  Each core runs the whole layer for
one batch element (B == 8 == n_cores), so there are no collectives.

Layout strategy per core:
  - LayerNorm in token-major layout (bn_stats over free dim), then the
    normalized tensor is transposed per 128x128 block on the PE; the LN
    affine (scale/bias, per feature == per partition after transpose) is
    fused into the PSUM eviction.
  - Q/K are produced feature-major [D, L]; V is produced feature-major then
    transposed into token-major V65 [L, 16*65] with a ones column per head
    (fused softmax denominator).
  - Scores are computed key-major S^T[keys, q] so softmax reduces over the
    PSUM partition dim via the matmul itself: exp is a single ACT op with
    the attention scale and the additive key mask folded into scale/bias,
    and O^T = V65^T @ expS accumulates both numerator and denominator.
  - The per-head division defers to a PE ones-broadcast of 1/denom plus one
    DVE multiply on [64, L].
  - MLP runs in two token halves so the gelu activations fit in SBUF.
  - All big matmuls run in float32r (full PE rate at N>=256, ~1e-4 rounding).

Weights are re-tiled on the host so every weight DMA is a contiguous
[128, 8, 128] (512 KB) transfer.
"""
from contextlib import ExitStack

import numpy as np

import concourse.bacc as bacc
import concourse.tile as tile
from concourse import mybir
from concourse.masks import make_identity

B, L, D = 8, 1024, 1024
H, HD, FF = 16, 64, 4096
EPS = 1e-5
P = 128
NCORES = 8
TC = L // P      # 8 token tiles
FC = D // P      # 8 feature tiles
MC = FF // P     # 32 ff tiles

f32 = mybir.dt.float32
f32r = mybir.dt.float32r
i32 = mybir.dt.int32
AF = mybir.ActivationFunctionType
ALU = mybir.AluOpType


def build_nc(replicas=1):
    """Build the Bass program. replicas>1 chains the layer body end-to-end
    through DRAM scratch (for timing slope measurements only)."""
    nc = bacc.Bacc(None)

    t = {}
    t["xd"] = nc.dram_tensor("x", [L, D], f32, kind="ExternalInput")
    t["maskd"] = nc.dram_tensor("mask", [L], i32, kind="ExternalInput")
    t["wqr"] = nc.dram_tensor("wqr", [FC, P, FC, P], f32r, kind="ExternalInput")
    t["wkr"] = nc.dram_tensor("wkr", [FC, P, FC, P], f32r, kind="ExternalInput")
    t["wvr"] = nc.dram_tensor("wvr", [FC, P, FC, P], f32r, kind="ExternalInput")
    t["wor"] = nc.dram_tensor("wor", [FC, P, FC, P], f32r, kind="ExternalInput")
    t["w1r"] = nc.dram_tensor("w1r", [MC, P, FC, P], f32r, kind="ExternalInput")
    t["w2r"] = nc.dram_tensor("w2r", [FC, P, MC, P], f32r, kind="ExternalInput")
    for nm, n in (("bq", D), ("bk", D), ("bv", D), ("bo", D), ("b1", FF),
                  ("b2", D), ("ln1_s", D), ("ln1_b", D), ("ln2_s", D),
                  ("ln2_b", D)):
        t[nm] = nc.dram_tensor(nm, [n], f32, kind="ExternalInput")
    t["yd"] = nc.dram_tensor("y", [L, D], f32, kind="ExternalOutput")

    with tile.TileContext(nc) as tc:
        with ExitStack() as ctx:
            pools = _make_pools(tc, ctx)
            consts = _emit_consts(nc, pools, t)
            x_src = t["xd"]
            for r in range(replicas):
                last = r == replicas - 1
                y_dst = t["yd"] if last else nc.dram_tensor(f"ychain{r}", [L, D], f32)
                x1_scr = nc.dram_tensor(f"x1_scratch{r}", [L, D], f32)
                _emit_layer(nc, pools, consts, t, x_src, y_dst, x1_scr)
                x_src = y_dst
    nc.compile()
    return nc


def _make_pools(tc, ctx):
    p = {}
    p["big"] = ctx.enter_context(tc.tile_pool(name="big", bufs=3))
    p["v65p"] = ctx.enter_context(tc.tile_pool(name="v65p", bufs=1))
    p["expp"] = ctx.enter_context(tc.tile_pool(name="expp", bufs=2))
    p["nst"] = ctx.enter_context(tc.tile_pool(name="nst", bufs=4))
    p["wp"] = ctx.enter_context(tc.tile_pool(name="wp", bufs=4))
    p["yp"] = ctx.enter_context(tc.tile_pool(name="yp", bufs=4))
    p["otp"] = ctx.enter_context(tc.tile_pool(name="otp", bufs=2))
    p["bcp"] = ctx.enter_context(tc.tile_pool(name="bcp", bufs=2))
    p["smal"] = ctx.enter_context(tc.tile_pool(name="smal", bufs=1))
    p["stat"] = ctx.enter_context(tc.tile_pool(name="stat", bufs=2))
    p["mm"] = ctx.enter_context(tc.tile_pool(name="mm", bufs=6, space="PSUM"))
    p["tpp"] = ctx.enter_context(tc.tile_pool(name="tpp", bufs=2, space="PSUM"))
    return p


def _emit_consts(nc, p, t):
    smal = p["smal"]
    c = {}

    ident_st = smal.tile([P, P], f32, tag="ident_st")
    make_identity(nc, ident_st[:])
    ident = smal.tile([P, P], f32r, tag="ident")
    nc.vector.tensor_copy(out=ident[:], in_=ident_st[:].bitcast(f32r))
    c["ident"] = ident

    ones_st = smal.tile([1, 64], f32, tag="ones_st")
    nc.vector.memset(ones_st[:], 1.0)
    ones_r = smal.tile([1, 64], f32r, tag="ones_r")
    nc.vector.tensor_copy(out=ones_r[:], in_=ones_st[:].bitcast(f32r))
    c["ones_r"] = ones_r

    def load_vec(name, n):
        tl = smal.tile([P, n // P], f32, tag=name + "t", name=name + "t")
        nc.gpsimd.dma_start(out=tl[:], in_=t[name][:].rearrange("(c p) -> p c", p=P))
        return tl

    for nm, n in (("bq", D), ("bk", D), ("bv", D), ("bo", D), ("b1", FF),
                  ("b2", D), ("ln1_s", D), ("ln1_b", D), ("ln2_s", D),
                  ("ln2_b", D)):
        c[nm] = load_vec(nm, n)

    epst = smal.tile([P, 1], f32, tag="epst")
    nc.vector.memset(epst[:], EPS)
    c["eps"] = epst

    # additive key mask: (m - 1) * 1e30  ->  0 or -1e30
    mi = smal.tile([P, TC], i32, tag="mi")
    nc.gpsimd.dma_start(out=mi[:], in_=t["maskd"][:].rearrange("(t p) -> p t", p=P))
    mf = smal.tile([P, TC], f32, tag="mf")
    nc.vector.tensor_copy(out=mf[:], in_=mi[:])
    fmask = smal.tile([P, TC], f32, tag="fmask")
    nc.vector.tensor_scalar(out=fmask[:], in0=mf[:], scalar1=1.0, scalar2=1e30,
                            op0=ALU.subtract, op1=ALU.mult)
    c["fmask"] = fmask

    ones_col_st = smal.tile([P, TC, H], f32, tag="ones_col_st")
    nc.vector.memset(ones_col_st[:], 1.0)
    c["ones_col_st"] = ones_col_st
    return c


def _emit_layer(nc, p, c, t, xd, yd, x1d):
    big, v65p, expp, nst = p["big"], p["v65p"], p["expp"], p["nst"]
    wp, yp, otp, bcp = p["wp"], p["yp"], p["otp"], p["bcp"]
    stat, mm, tpp = p["stat"], p["mm"], p["tpp"]
    ident, ones_r, fmask = c["ident"], c["ones_r"], c["fmask"]

    # V65: token-major V with a ones column per head
    v65 = v65p.tile([P, TC, H * 65], f32r, tag="v65", name="v65")
    v65_ones = v65[:, :, :].rearrange("p t (h c) -> p t h c", c=65)[:, :, :, 64]
    nc.vector.tensor_copy(out=v65_ones, in_=c["ones_col_st"][:].bitcast(f32r))

    def layernorm_tile(x_tc):
        """token-major [128, D] -> normalized f32r tile (no affine)."""
        st = stat.tile([P, 2, nc.vector.BN_STATS_DIM], f32, tag="bnst", name="st")
        xg = x_tc[:].rearrange("p (s f) -> p s f", s=2)
        for s in range(2):
            nc.vector.bn_stats(out=st[:, s, :], in_=xg[:, s, :])
        mv = stat.tile([P, nc.vector.BN_AGGR_DIM], f32, tag="bnmv", name="mv")
        nc.vector.bn_aggr(out=mv[:], in_=st[:])
        sd = stat.tile([P, 1], f32, tag="bnsd", name="sd")
        nc.scalar.activation(sd[:], mv[:, 1:2], AF.Sqrt, bias=c["eps"][:], scale=1.0)
        r0 = stat.tile([P, 1], f32, tag="bnr0", name="r0")
        nc.vector.reciprocal(out=r0[:], in_=sd[:])
        # one Newton step: r1 = r0 * (2 - sd*r0)
        t1 = stat.tile([P, 1], f32, tag="bnt1", name="t1")
        nc.vector.tensor_mul(t1[:], sd[:], r0[:])
        nc.vector.tensor_scalar(out=t1[:], in0=t1[:], scalar1=-1.0, scalar2=2.0,
                                op0=ALU.mult, op1=ALU.add)
        rstd = stat.tile([P, 1], f32, tag="bnrstd", name="rstd")
        nc.vector.tensor_mul(rstd[:], r0[:], t1[:])
        n_tc = nst.tile([P, D], f32r, tag="nstage", name="n_tc")
        nc.vector.tensor_scalar(out=n_tc[:], in0=x_tc[:], scalar1=mv[:, 0:1],
                                scalar2=rstd[:], op0=ALU.subtract, op1=ALU.mult)
        return n_tc

    # ---------------- LN1 -> h1T (feature-major, f32r) ----------------
    h1T = big.tile([P, FC, L], f32r, tag="big", name="h1T")
    for tt in range(TC):
        x_tc = nst.tile([P, D], f32, tag="nstage", name="x_tc")
        nc.sync.dma_start(out=x_tc[:], in_=xd[tt * P:(tt + 1) * P, :])
        n_tc = layernorm_tile(x_tc)
        for cc in range(FC):
            tp = tpp.tile([P, P], f32r, tag="tp", name="tpln1")
            nc.tensor.transpose(tp[:], n_tc[:, cc * P:(cc + 1) * P], ident[:])
            nc.scalar.activation(h1T[:, cc, tt * P:(tt + 1) * P], tp[:].bitcast(f32),
                                 AF.Identity, bias=c["ln1_b"][:, cc:cc + 1],
                                 scale=c["ln1_s"][:, cc:cc + 1])

    # ---------------- Q/K projections (feature-major) ----------------
    qT = big.tile([P, FC, L], f32r, tag="big", name="qT")
    kT = big.tile([P, FC, L], f32r, tag="big", name="kT")
    for dst, wsrc, bias in ((qT, t["wqr"], c["bq"]), (kT, t["wkr"], c["bk"])):
        for fc in range(FC):
            wt = wp.tile([P, FC, P], f32r, tag="w", name="wqk")
            nc.sync.dma_start(out=wt[:], in_=wsrc[fc])
            for half in range(2):
                ps = mm.tile([P, 512], f32, tag="mm", name="psqk")
                for kt in range(FC):
                    nc.tensor.matmul(
                        ps[:], wt[:, kt, :],
                        h1T[:, kt, half * 512:(half + 1) * 512],
                        start=(kt == 0), stop=(kt == FC - 1))
                nc.vector.tensor_scalar_add(
                    out=dst[:, fc, half * 512:(half + 1) * 512],
                    in0=ps[:].bitcast(f32r), scalar1=bias[:, fc:fc + 1])

    # ---------------- V projection -> V65 (token-major + ones) -------
    for fc in range(FC):
        wt = wp.tile([P, FC, P], f32r, tag="w", name="wv")
        nc.sync.dma_start(out=wt[:], in_=t["wvr"][fc])
        vt_fc = nst.tile([P, L], f32r, tag="nstage", name="vt_fc")
        for half in range(2):
            ps = mm.tile([P, 512], f32, tag="mm", name="psv")
            for kt in range(FC):
                nc.tensor.matmul(
                    ps[:], wt[:, kt, :],
                    h1T[:, kt, half * 512:(half + 1) * 512],
                    start=(kt == 0), stop=(kt == FC - 1))
            nc.vector.tensor_scalar_add(
                out=vt_fc[:, half * 512:(half + 1) * 512],
                in0=ps[:].bitcast(f32r), scalar1=c["bv"][:, fc:fc + 1])
        # transpose [vfeat, tok] blocks into token-major V65 homes
        for tt in range(TC):
            tp = tpp.tile([P, P], f32r, tag="tp", name="tpv")
            nc.tensor.transpose(tp[:], vt_fc[:, tt * P:(tt + 1) * P], ident[:])
            dst = v65[:, tt, :].rearrange("p (h c) -> p h c", c=65)[:, 2 * fc:2 * fc + 2, :64]
            nc.vector.tensor_copy(out=dst, in_=tp[:].rearrange("p (h c) -> p h c", c=64))

    # ---------------- attention ----------------
    attnT = big.tile([P, FC, L], f32r, tag="big", name="attnT")
    for h in range(H):
        p0 = (h % 2) * 64
        hc = h // 2
        ot_ps = [mm.tile([65, 512], f32, tag="mm", name=f"ot_ps{h}_{i}")
                 for i in range(2)]
        for kt in range(TC):
            st_ps = [mm.tile([P, 512], f32, tag="mm", name=f"st_ps{h}_{kt}_{i}")
                     for i in range(2)]
            for half in range(2):
                nc.tensor.matmul(
                    st_ps[half][:],
                    kT[p0:p0 + 64, hc, kt * P:(kt + 1) * P],
                    qT[p0:p0 + 64, hc, half * 512:(half + 1) * 512],
                    start=True, stop=True)
            es = expp.tile([P, L], f32r, tag="expS", name="es")
            for half in range(2):
                nc.scalar.activation(
                    es[:, half * 512:(half + 1) * 512], st_ps[half][:],
                    AF.Exp, bias=fmask[:, kt:kt + 1], scale=0.125)
            for half in range(2):
                nc.tensor.matmul(
                    ot_ps[half][:],
                    v65[:, kt, h * 65:(h + 1) * 65],
                    es[:, half * 512:(half + 1) * 512],
                    start=(kt == 0), stop=(kt == TC - 1))
        # epilogue: 1/denom (+1 Newton step), PE broadcast, divide
        for half in range(2):
            r0 = stat.tile([1, 512], f32, tag="hr0", name="hr0")
            nc.vector.reciprocal(out=r0[:], in_=ot_ps[half][64:65, :])
            t1 = stat.tile([1, 512], f32, tag="ht1", name="ht1")
            nc.vector.tensor_mul(t1[:], ot_ps[half][64:65, :], r0[:])
            nc.vector.tensor_scalar(out=t1[:], in0=t1[:], scalar1=-1.0,
                                    scalar2=2.0, op0=ALU.mult, op1=ALU.add)
            rec = stat.tile([1, 512], f32r, tag="hrec", name="hrec")
            nc.vector.tensor_mul(rec[:], r0[:], t1[:])
            bc_ps = tpp.tile([64, 512], f32, tag="tp", name="bc_ps")
            nc.tensor.matmul(bc_ps[:], ones_r[:], rec[:], start=True, stop=True)
            bc = bcp.tile([64, 512], f32, tag="bc", name="bc")
            nc.scalar.activation(bc[:], bc_ps[:], AF.Copy)
            if p0 == 0:
                nc.vector.tensor_mul(
                    attnT[0:64, hc, half * 512:(half + 1) * 512],
                    ot_ps[half][0:64, :].bitcast(f32r), bc[:].bitcast(f32r))
            else:
                od = otp.tile([64, 512], f32r, tag="otdiv", name="od")
                nc.vector.tensor_mul(od[:], ot_ps[half][0:64, :].bitcast(f32r),
                                     bc[:].bitcast(f32r))
                nc.gpsimd.dma_start(
                    out=attnT[p0:p0 + 64, hc, half * 512:(half + 1) * 512],
                    in_=od[:])

    # ---------------- out projection -> ZT (feature-major) ----------
    zT = big.tile([P, FC, L], f32r, tag="big", name="zT")
    for m in range(FC):
        wt = wp.tile([P, FC, P], f32r, tag="w", name="wo")
        nc.sync.dma_start(out=wt[:], in_=t["wor"][m])
        for half in range(2):
            ps = mm.tile([P, 512], f32, tag="mm", name="psz")
            for ks in range(FC):
                nc.tensor.matmul(
                    ps[:], wt[:, ks, :],
                    attnT[:, ks, half * 512:(half + 1) * 512],
                    start=(ks == 0), stop=(ks == FC - 1))
            nc.vector.tensor_scalar_add(
                out=zT[:, m, half * 512:(half + 1) * 512],
                in0=ps[:].bitcast(f32r), scalar1=c["bo"][:, m:m + 1])

    # -------- residual 1 + LN2 -> h2T, x1 -> DRAM scratch ------------
    h2T = big.tile([P, FC, L], f32r, tag="big", name="h2T")
    for tt in range(TC):
        x1_tc = nst.tile([P, D], f32, tag="nstage", name="x1_tc")
        nc.sync.dma_start(out=x1_tc[:], in_=xd[tt * P:(tt + 1) * P, :])
        for cc in range(FC):
            tp = tpp.tile([P, P], f32r, tag="tp", name="tpz")
            nc.tensor.transpose(tp[:], zT[:, cc, tt * P:(tt + 1) * P], ident[:])
            nc.vector.tensor_add(x1_tc[:, cc * P:(cc + 1) * P],
                                 x1_tc[:, cc * P:(cc + 1) * P],
                                 tp[:].bitcast(f32))
        nc.sync.dma_start(out=x1d[tt * P:(tt + 1) * P, :], in_=x1_tc[:])
        n_tc = layernorm_tile(x1_tc)
        for cc in range(FC):
            tp = tpp.tile([P, P], f32r, tag="tp", name="tpln2")
            nc.tensor.transpose(tp[:], n_tc[:, cc * P:(cc + 1) * P], ident[:])
            nc.scalar.activation(h2T[:, cc, tt * P:(tt + 1) * P], tp[:].bitcast(f32),
                                 AF.Identity, bias=c["ln2_b"][:, cc:cc + 1],
                                 scale=c["ln2_s"][:, cc:cc + 1])

    # ---------------- MLP in two token halves ----------------
    for half in range(2):
        g0 = big.tile([P, 16, 512], f32r, tag="big", name=f"g0_{half}")
        g1 = big.tile([P, 16, 512], f32r, tag="big", name=f"g1_{half}")
        gs = (g0, g1)
        for m in range(MC):
            wt = wp.tile([P, FC, P], f32r, tag="w", name="w1t")
            nc.sync.dma_start(out=wt[:], in_=t["w1r"][m])
            ps = mm.tile([P, 512], f32, tag="mm", name="psf1")
            for kt in range(FC):
                nc.tensor.matmul(
                    ps[:], wt[:, kt, :],
                    h2T[:, kt, half * 512:(half + 1) * 512],
                    start=(kt == 0), stop=(kt == FC - 1))
            nc.scalar.activation(gs[m // 16][:, m % 16, :], ps[:],
                                 AF.Gelu_apprx_sigmoid,
                                 bias=c["b1"][:, m:m + 1], scale=1.0)
        # y tiles for this half, initialized from x1
        y_tcs = []
        for tq in range(4):
            tt = half * 4 + tq
            y_tc = yp.tile([P, D], f32, tag="y", name=f"y_tc{half}_{tq}")
            nc.sync.dma_start(out=y_tc[:], in_=x1d[tt * P:(tt + 1) * P, :])
            y_tcs.append(y_tc)
        for m in range(FC):
            ps = mm.tile([P, 512], f32, tag="mm", name=f"psf2_{half}_{m}")
            for g in range(4):
                wt = wp.tile([P, FC, P], f32r, tag="w", name="w2t")
                nc.sync.dma_start(out=wt[:], in_=t["w2r"][m, :, g * 8:(g + 1) * 8, :])
                for kt in range(FC):
                    ktg = g * 8 + kt
                    nc.tensor.matmul(
                        ps[:], wt[:, kt, :],
                        gs[ktg // 16][:, ktg % 16, :],
                        start=(ktg == 0), stop=(ktg == MC - 1))
            mt = nst.tile([P, 512], f32, tag="nstage", name="mt")
            nc.vector.tensor_scalar_add(out=mt[:], in0=ps[:],
                                        scalar1=c["b2"][:, m:m + 1])
            mtr = nst.tile([P, 512], f32r, tag="nstage", name="mtr")
            nc.vector.tensor_copy(out=mtr[:], in_=mt[:].bitcast(f32r))
            for tq in range(4):
                tp = tpp.tile([P, P], f32r, tag="tp", name="tpmt")
                nc.tensor.transpose(tp[:], mtr[:, tq * P:(tq + 1) * P], ident[:])
                nc.vector.tensor_add(y_tcs[tq][:, m * P:(m + 1) * P],
                                     y_tcs[tq][:, m * P:(m + 1) * P],
                                     tp[:].bitcast(f32))
        for tq in range(4):
            tt = half * 4 + tq
            nc.sync.dma_start(out=yd[tt * P:(tt + 1) * P, :], in_=y_tcs[tq][:])


_NC_CACHE = {}


def _get_nc(replicas=1):
    if replicas not in _NC_CACHE:
        _NC_CACHE[replicas] = build_nc(replicas)
    return _NC_CACHE[replicas]


def _retile(w, kslices, mslices):
    """[K, M] -> [mslices, 128, kslices, 128], blk[m,p,k,c] = w[k*128+p, m*128+c]."""
    K, M = w.shape
    assert K == kslices * P and M == mslices * P
    return np.ascontiguousarray(
        w.reshape(kslices, P, mslices, P).transpose(2, 1, 0, 3))


def make_in_maps(x, attention_mask, wq, bq, wk, bk, wv, bv, wo, bo,
                 ln1_s, ln1_b, ln2_s, ln2_b, w1, b1, w2, b2):
    f = np.asarray
    shared = {
        "wqr": _retile(f(wq, dtype=np.float32), FC, FC),
        "wkr": _retile(f(wk, dtype=np.float32), FC, FC),
        "wvr": _retile(f(wv, dtype=np.float32), FC, FC),
        "wor": _retile(f(wo, dtype=np.float32), FC, FC),
        "w1r": _retile(f(w1, dtype=np.float32), FC, MC),
        "w2r": _retile(f(w2, dtype=np.float32), MC, FC),
        "bq": f(bq, dtype=np.float32), "bk": f(bk, dtype=np.float32),
        "bv": f(bv, dtype=np.float32), "bo": f(bo, dtype=np.float32),
        "b1": f(b1, dtype=np.float32), "b2": f(b2, dtype=np.float32),
        "ln1_s": f(ln1_s, dtype=np.float32), "ln1_b": f(ln1_b, dtype=np.float32),
        "ln2_s": f(ln2_s, dtype=np.float32), "ln2_b": f(ln2_b, dtype=np.float32),
    }
    x = f(x, dtype=np.float32)
    m = f(attention_mask, dtype=np.int32)
    return [dict(shared, x=np.ascontiguousarray(x[c]),
                 mask=np.ascontiguousarray(m[c])) for c in range(NCORES)]


def kernel(**inputs):
    from concourse.bass_utils import run_bass_kernel_spmd

    nc = _get_nc()
    in_maps = make_in_maps(**inputs)
    res = run_bass_kernel_spmd(nc, in_maps, core_ids=list(range(NCORES)))
    out = np.stack([res.results[c]["y"] for c in range(NCORES)], axis=0)
    return out.astype(np.float32)


# revision 18
# speedup vs baseline: 13.3302x; 1.5163x over previous
"""CLIP encoder layer on 8 Trainium2 NeuronCores, data-parallel over batch.

Full (unsharded) inputs -> full output.  Each core runs the whole layer for
one batch element (B == 8 == n_cores), so there are no collectives.

Layout strategy per core:
  - LayerNorm in token-major layout (bn_stats over free dim), then the
    normalized tensor is transposed per 128x128 block on the PE; the LN
    affine (scale/bias, per feature == per partition after transpose) is
    fused into the PSUM eviction.
  - Q/K are produced feature-major [D, L]; V is produced feature-major then
    transposed into token-major V65 [L, 16*65] with a ones column per head
    (fused softmax denominator).
  - Scores are computed key-major S^T[keys, q] so softmax reduces over the
    PSUM partition dim via the matmul itself: exp is a single ACT op with
    the attention scale and the additive key mask folded into scale/bias,
    and O^T = V65^T @ expS accumulates both numerator and denominator.
  - The per-head division defers to a PE ones-broadcast of 1/denom plus one
    DVE multiply on [64, L].
  - MLP runs in two token halves so the gelu activations fit in SBUF.
  - All big matmuls run in float32r (full PE rate at N>=256, ~1e-4 rounding).

Weights are re-tiled on the host so every weight DMA is a contiguous
[128, 8, 128] (512 KB) transfer.
"""
from contextlib import ExitStack

import numpy as np

import concourse.bacc as bacc
import concourse.tile as tile
from concourse import mybir
from concourse.masks import make_identity

B, L, D = 8, 1024, 1024
H, HD, FF = 16, 64, 4096
EPS = 1e-5
P = 128
NCORES = 8
TC = L // P      # 8 token tiles
FC = D // P      # 8 feature tiles
MC = FF // P     # 32 ff tiles

f32 = mybir.dt.float32
f32r = mybir.dt.float32r
i32 = mybir.dt.int32
AF = mybir.ActivationFunctionType
ALU = mybir.AluOpType


def build_nc(replicas=1):
    """Build the Bass program. replicas>1 chains the layer body end-to-end
    through DRAM scratch (for timing slope measurements only)."""
    nc = bacc.Bacc(None)

    t = {}
    t["xd"] = nc.dram_tensor("x", [L, D], f32, kind="ExternalInput")
    t["maskd"] = nc.dram_tensor("mask", [L], i32, kind="ExternalInput")
    t["wqr"] = nc.dram_tensor("wqr", [FC, P, FC, P], f32r, kind="ExternalInput")
    t["wkr"] = nc.dram_tensor("wkr", [FC, P, FC, P], f32r, kind="ExternalInput")
    t["wvr"] = nc.dram_tensor("wvr", [FC, P, FC, P], f32r, kind="ExternalInput")
    t["wor"] = nc.dram_tensor("wor", [FC, P, FC, P], f32r, kind="ExternalInput")
    t["w1r"] = nc.dram_tensor("w1r", [MC, P, FC, P], f32r, kind="ExternalInput")
    t["w2r"] = nc.dram_tensor("w2r", [FC, P, MC, P], f32r, kind="ExternalInput")
    for nm, n in (("bq", D), ("bk", D), ("bv", D), ("bo", D), ("b1", FF),
                  ("b2", D)):
        t[nm] = nc.dram_tensor(nm, [n], f32, kind="ExternalInput")
    t["yd"] = nc.dram_tensor("y", [L, D], f32, kind="ExternalOutput")

    with tile.TileContext(nc) as tc:
        with ExitStack() as ctx:
            pools = _make_pools(tc, ctx)
            consts = _emit_consts(nc, pools, t)
            x_src = t["xd"]
            for r in range(replicas):
                last = r == replicas - 1
                y_dst = t["yd"] if last else nc.dram_tensor(f"ychain{r}", [L, D], f32)
                x1_scr = nc.dram_tensor(f"x1_scratch{r}", [L, D], f32)
                rec_scr = nc.dram_tensor(f"rec_scratch{r}", [H, L], f32)
                _emit_layer(nc, pools, consts, t, x_src, y_dst, x1_scr, rec_scr)
                x_src = y_dst
    nc.compile()
    return nc


def _make_pools(tc, ctx):
    p = {}
    p["big"] = ctx.enter_context(tc.tile_pool(name="big", bufs=3))
    p["v65p"] = ctx.enter_context(tc.tile_pool(name="v65p", bufs=1))
    p["expp"] = ctx.enter_context(tc.tile_pool(name="expp", bufs=3))
    p["nst"] = ctx.enter_context(tc.tile_pool(name="nst", bufs=4))
    p["wp"] = ctx.enter_context(tc.tile_pool(name="wp", bufs=4))
    p["yp"] = ctx.enter_context(tc.tile_pool(name="yp", bufs=4))
    p["otp"] = ctx.enter_context(tc.tile_pool(name="otp", bufs=1))
    p["bcp"] = ctx.enter_context(tc.tile_pool(name="bcp", bufs=3))
    p["smal"] = ctx.enter_context(tc.tile_pool(name="smal", bufs=1))
    p["stat"] = ctx.enter_context(tc.tile_pool(name="stat", bufs=2))
    p["mmS"] = ctx.enter_context(tc.tile_pool(name="mmS", bufs=2, space="PSUM"))
    p["mmO"] = ctx.enter_context(tc.tile_pool(name="mmO", bufs=2, space="PSUM"))
    return p


def _emit_consts(nc, p, t):
    smal = p["smal"]
    c = {}

    ident_st = smal.tile([P, P], f32, tag="ident_st")
    make_identity(nc, ident_st[:])
    ident = smal.tile([P, P], f32r, tag="ident")
    nc.vector.tensor_copy(out=ident[:], in_=ident_st[:].bitcast(f32r))
    c["ident"] = ident

    def load_vec(name, n):
        tl = smal.tile([P, n // P], f32, tag=name + "t", name=name + "t")
        nc.gpsimd.dma_start(out=tl[:], in_=t[name][:].rearrange("(c p) -> p c", p=P))
        return tl

    for nm, n in (("bq", D), ("bk", D), ("bv", D), ("bo", D), ("b1", FF),
                  ("b2", D)):
        c[nm] = load_vec(nm, n)

    epst = smal.tile([P, 1], f32, tag="epst")
    nc.vector.memset(epst[:], EPS)
    c["eps"] = epst

    # additive key mask: (m - 1) * 1e30  ->  0 or -1e30
    mi = smal.tile([P, TC], i32, tag="mi")
    nc.gpsimd.dma_start(out=mi[:], in_=t["maskd"][:].rearrange("(t p) -> p t", p=P))
    mf = smal.tile([P, TC], f32, tag="mf")
    nc.vector.tensor_copy(out=mf[:], in_=mi[:])
    fmask = smal.tile([P, TC], f32, tag="fmask")
    nc.vector.tensor_scalar(out=fmask[:], in0=mf[:], scalar1=1.0, scalar2=1e30,
                            op0=ALU.subtract, op1=ALU.mult)
    c["fmask"] = fmask

    ones_col_st = smal.tile([P, TC, H], f32, tag="ones_col_st")
    nc.vector.memset(ones_col_st[:], 1.0)
    c["ones_col_st"] = ones_col_st
    return c


def _emit_layer(nc, p, c, t, xd, yd, x1d, recd):
    big, v65p, expp, nst = p["big"], p["v65p"], p["expp"], p["nst"]
    wp, yp, otp, bcp = p["wp"], p["yp"], p["otp"], p["bcp"]
    stat, mmS, mmO, smal = p["stat"], p["mmS"], p["mmO"], p["smal"]
    ident, fmask = c["ident"], c["fmask"]
    import concourse.bass as bass

    # V65: token-major V with a ones column per head
    v65 = v65p.tile([P, TC, H * 65], f32r, tag="v65", name="v65")
    v65_ones = v65[:, :, :].rearrange("p t (h c) -> p t h c", c=65)[:, :, :, 64]
    nc.vector.tensor_copy(out=v65_ones, in_=c["ones_col_st"][:].bitcast(f32r))

    def layernorm_tile(x_tc):
        """token-major [128, D] -> normalized f32r tile (affine folded into
        the following projection weights on the host)."""
        st = stat.tile([P, 2, nc.vector.BN_STATS_DIM], f32, tag="bnst", name="st")
        xg = x_tc[:].rearrange("p (s f) -> p s f", s=2)
        for s in range(2):
            nc.vector.bn_stats(out=st[:, s, :], in_=xg[:, s, :])
        mv = stat.tile([P, nc.vector.BN_AGGR_DIM], f32, tag="bnmv", name="mv")
        nc.vector.bn_aggr(out=mv[:], in_=st[:])
        sd = stat.tile([P, 1], f32, tag="bnsd", name="sd")
        nc.scalar.activation(sd[:], mv[:, 1:2], AF.Sqrt, bias=c["eps"][:], scale=1.0)
        r0 = stat.tile([P, 1], f32, tag="bnr0", name="r0")
        nc.vector.reciprocal(out=r0[:], in_=sd[:])
        # one Newton step: r1 = r0 * (2 - sd*r0)
        t1 = stat.tile([P, 1], f32, tag="bnt1", name="t1")
        nc.vector.tensor_mul(t1[:], sd[:], r0[:])
        nc.vector.tensor_scalar(out=t1[:], in0=t1[:], scalar1=-1.0, scalar2=2.0,
                                op0=ALU.mult, op1=ALU.add)
        rstd = stat.tile([P, 1], f32, tag="bnrstd", name="rstd")
        nc.vector.tensor_mul(rstd[:], r0[:], t1[:])
        n_tc = nst.tile([P, D], f32r, tag="nstage", name="n_tc")
        nc.vector.tensor_scalar(out=n_tc[:], in0=x_tc[:], scalar1=mv[:, 0:1],
                                scalar2=rstd[:], op0=ALU.subtract, op1=ALU.mult)
        return n_tc

    def transpose_to(dst_ap_fn, n_tc, tt, evict="act"):
        """Transpose [128, D] token-major tile into feature-major homes,
        grouped 4 feature-blocks per PSUM tile / eviction op."""
        for cg in range(2):
            tp = mmO.tile([P, 512], f32r, tag="mmO", name="tpg")
            for j in range(4):
                cc = cg * 4 + j
                nc.tensor.transpose(tp[:, j * P:(j + 1) * P],
                                    n_tc[:, cc * P:(cc + 1) * P], ident[:])
            dst = dst_ap_fn(cg)
            if evict == "act":
                nc.scalar.activation(dst, tp[:].bitcast(f32).rearrange(
                    "p (j q) -> p j q", j=4), AF.Copy)
            else:
                nc.vector.tensor_copy(out=dst, in_=tp[:].rearrange(
                    "p (j q) -> p j q", j=4))

    # ---------------- LN1 -> h1T (feature-major, f32r) ----------------
    h1T = big.tile([P, FC, L], f32r, tag="big", name="h1T")
    prev_ln1 = None
    for tt in range(TC):
        x_tc = nst.tile([P, D], f32, tag="nstage", name="x_tc")
        nc.sync.dma_start(out=x_tc[:], in_=xd[tt * P:(tt + 1) * P, :])
        if prev_ln1 is not None:
            pn, ptt = prev_ln1
            transpose_to(
                lambda cg: h1T[:, cg * 4:(cg + 1) * 4, ptt * P:(ptt + 1) * P],
                pn, ptt, evict="act")
        n_tc = layernorm_tile(x_tc)
        prev_ln1 = (n_tc, tt)
    pn, ptt = prev_ln1
    transpose_to(
        lambda cg: h1T[:, cg * 4:(cg + 1) * 4, ptt * P:(ptt + 1) * P],
        pn, ptt, evict="act")

    # ---------------- Q/K projections (feature-major) ----------------
    qT = big.tile([P, FC, L], f32r, tag="big", name="qT")
    kT = big.tile([P, FC, L], f32r, tag="big", name="kT")
    for dst, wsrc, bias in ((qT, t["wqr"], c["bq"]), (kT, t["wkr"], c["bk"])):
        for fc in range(FC):
            wt = wp.tile([P, FC, P], f32r, tag="w", name="wqk")
            nc.sync.dma_start(out=wt[:], in_=wsrc[fc])
            ps = mmS.tile([P, L], f32, tag="mmS", name="psqk")
            for half in range(2):
                for kt in range(FC):
                    nc.tensor.matmul(
                        ps[:, half * 512:(half + 1) * 512], wt[:, kt, :],
                        h1T[:, kt, half * 512:(half + 1) * 512],
                        start=(kt == 0), stop=(kt == FC - 1))
            nc.vector.tensor_scalar_add(
                out=dst[:, fc, :], in0=ps[:].bitcast(f32r),
                scalar1=bias[:, fc:fc + 1])

    # ---------------- V projection -> V65 (token-major + ones) -------
    for fc in range(FC):
        wt = wp.tile([P, FC, P], f32r, tag="w", name="wv")
        nc.sync.dma_start(out=wt[:], in_=t["wvr"][fc])
        vt_fc = nst.tile([P, L], f32r, tag="nstage", name="vt_fc")
        ps = mmS.tile([P, L], f32, tag="mmS", name="psv")
        for half in range(2):
            for kt in range(FC):
                nc.tensor.matmul(
                    ps[:, half * 512:(half + 1) * 512], wt[:, kt, :],
                    h1T[:, kt, half * 512:(half + 1) * 512],
                    start=(kt == 0), stop=(kt == FC - 1))
        nc.vector.tensor_scalar_add(
            out=vt_fc[:], in0=ps[:].bitcast(f32r), scalar1=c["bv"][:, fc:fc + 1])
        # transpose [vfeat, tok] blocks into token-major V65 homes (4 tc/group)
        for tg in range(2):
            tp = mmO.tile([P, 512], f32r, tag="mmO", name="tpvg")
            for j in range(4):
                tt = tg * 4 + j
                nc.tensor.transpose(tp[:, j * P:(j + 1) * P],
                                    vt_fc[:, tt * P:(tt + 1) * P], ident[:])
            dst = v65[:, tg * 4:(tg + 1) * 4, :].rearrange(
                "p t (h q) -> p t h q", q=65)[:, :, 2 * fc:2 * fc + 2, :64]
            nc.vector.tensor_copy(
                out=dst, in_=tp[:].rearrange("p (t h q) -> p t h q", t=4, h=2))

    # ---------------- attention ----------------
    # Software-pipelined: S^T(kt+1) is emitted before O^T(kt) so the PE never
    # stalls on the ACT exp; head h-1's epilogue is tucked under head h's
    # first two score tiles.  The 1/denom broadcast goes through a DRAM
    # scratch row (DMA partition-broadcast) so it costs no PE/PSUM.
    attnT = big.tile([P, FC, L], f32r, tag="big", name="attnT")
    wo_pf = []
    for m in range(2):
        wt = wp.tile([P, FC, P], f32r, tag="w", name=f"wo_pf{m}")
        nc.sync.dma_start(out=wt[:], in_=t["wor"][m])
        wo_pf.append(wt)

    def head_epilogue(h, ot_ps):
        p0 = (h % 2) * 64
        hc = h // 2
        r0 = bcp.tile([1, L], f32, tag="bc", name="hr0")
        nc.vector.reciprocal(out=r0[:], in_=ot_ps[64:65, :])
        t1 = bcp.tile([1, L], f32, tag="bc", name="ht1")
        nc.vector.tensor_mul(t1[:], ot_ps[64:65, :], r0[:])
        nc.vector.tensor_scalar(out=t1[:], in0=t1[:], scalar1=-1.0,
                                scalar2=2.0, op0=ALU.mult, op1=ALU.add)
        nc.vector.tensor_mul(r0[:], r0[:], t1[:])
        nc.gpsimd.dma_start(out=recd[h:h + 1, :], in_=r0[:])
        bc = bcp.tile([64, L], f32, tag="bc", name="bc")
        bcast_src = bass.AP(tensor=recd, offset=h * L,
                            ap=[[0, 64], [1, L]])
        nc.gpsimd.dma_start(out=bc[:], in_=bcast_src)
        if p0 == 0:
            nc.vector.tensor_mul(
                attnT[0:64, hc, :],
                ot_ps[0:64, :].bitcast(f32r), bc[:].bitcast(f32r))
        else:
            od = otp.tile([64, L], f32r, tag="otdiv", name="od")
            nc.vector.tensor_mul(od[:], ot_ps[0:64, :].bitcast(f32r),
                                 bc[:].bitcast(f32r))
            nc.gpsimd.dma_start(out=attnT[p0:p0 + 64, hc, :], in_=od[:])

    prev_h = None
    prev_ot = None
    head_order = [h for h in range(H) if h % 2 == 1] + \
                 [h for h in range(H) if h % 2 == 0]
    for h in head_order:
        p0 = (h % 2) * 64
        hc = h // 2
        ess = []

        def score_tile(kt, h=h, p0=p0, hc=hc, ess=ess):
            st_ps = mmS.tile([P, L], f32, tag="mmS", name=f"st_ps{h}_{kt}")
            for half in range(2):
                nc.tensor.matmul(
                    st_ps[:, half * 512:(half + 1) * 512],
                    kT[p0:p0 + 64, hc, kt * P:(kt + 1) * P],
                    qT[p0:p0 + 64, hc, half * 512:(half + 1) * 512],
                    start=True, stop=True)
            es = expp.tile([P, L], f32r, tag="expS", name="es")
            nc.scalar.activation(es[:], st_ps[:], AF.Exp,
                                 bias=fmask[:, kt:kt + 1], scale=0.125)
            ess.append(es)

        score_tile(0)
        score_tile(1)
        if prev_ot is not None:
            head_epilogue(prev_h, prev_ot)
        ot_ps = mmO.tile([65, L], f32, tag="mmO", name=f"ot_ps{h}")
        for kt in range(TC):
            if kt + 2 < TC:
                score_tile(kt + 2)
            es = ess[kt]
            for half in range(2):
                nc.tensor.matmul(
                    ot_ps[:, half * 512:(half + 1) * 512],
                    v65[:, kt, h * 65:(h + 1) * 65],
                    es[:, half * 512:(half + 1) * 512],
                    start=(kt == 0), stop=(kt == TC - 1))
        prev_ot = ot_ps
        prev_h = h
    head_epilogue(prev_h, prev_ot)

    # ---------------- out projection -> ZT (feature-major) ----------
    zT = big.tile([P, FC, L], f32r, tag="big", name="zT")
    for m in range(FC):
        if m < 2:
            wt = wo_pf[m]
        else:
            wt = wp.tile([P, FC, P], f32r, tag="w", name="wo")
            nc.sync.dma_start(out=wt[:], in_=t["wor"][m])
        ps = mmS.tile([P, L], f32, tag="mmS", name="psz")
        for half in range(2):
            for ks in range(FC):
                nc.tensor.matmul(
                    ps[:, half * 512:(half + 1) * 512], wt[:, ks, :],
                    attnT[:, ks, half * 512:(half + 1) * 512],
                    start=(ks == 0), stop=(ks == FC - 1))
        nc.vector.tensor_scalar_add(
            out=zT[:, m, :], in0=ps[:].bitcast(f32r), scalar1=c["bo"][:, m:m + 1])

    # -------- residual 1 + LN2 -> h2T, x1 -> DRAM scratch ------------
    h2T = big.tile([P, FC, L], f32r, tag="big", name="h2T")
    prev_ln2 = None
    for tt in range(TC):
        x1_tc = nst.tile([P, D], f32, tag="nstage", name="x1_tc")
        nc.sync.dma_start(out=x1_tc[:], in_=xd[tt * P:(tt + 1) * P, :])
        for cg in range(2):
            tp = mmO.tile([P, 512], f32r, tag="mmO", name="tpzg")
            for j in range(4):
                cc = cg * 4 + j
                nc.tensor.transpose(tp[:, j * P:(j + 1) * P],
                                    zT[:, cc, tt * P:(tt + 1) * P], ident[:])
            nc.vector.tensor_add(x1_tc[:, cg * 512:(cg + 1) * 512],
                                 x1_tc[:, cg * 512:(cg + 1) * 512],
                                 tp[:].bitcast(f32))
        nc.sync.dma_start(out=x1d[tt * P:(tt + 1) * P, :], in_=x1_tc[:])
        if prev_ln2 is not None:
            pn, ptt = prev_ln2
            transpose_to(
                lambda cg: h2T[:, cg * 4:(cg + 1) * 4, ptt * P:(ptt + 1) * P],
                pn, ptt, evict="act")
        n_tc = layernorm_tile(x1_tc)
        prev_ln2 = (n_tc, tt)
    pn, ptt = prev_ln2
    transpose_to(
        lambda cg: h2T[:, cg * 4:(cg + 1) * 4, ptt * P:(ptt + 1) * P],
        pn, ptt, evict="act")

    # ---------------- MLP in two token halves ----------------
    for half in range(2):
        g0 = big.tile([P, 16, 512], f32r, tag="big", name=f"g0_{half}")
        g1 = big.tile([P, 16, 512], f32r, tag="big", name=f"g1_{half}")
        gs = (g0, g1)
        for m in range(MC):
            wt = wp.tile([P, FC, P], f32r, tag="w", name="w1t")
            nc.sync.dma_start(out=wt[:], in_=t["w1r"][m])
            ps = mmS.tile([P, 512], f32, tag="mmS", name="psf1")
            for kt in range(FC):
                nc.tensor.matmul(
                    ps[:], wt[:, kt, :],
                    h2T[:, kt, half * 512:(half + 1) * 512],
                    start=(kt == 0), stop=(kt == FC - 1))
            nc.scalar.activation(gs[m // 16][:, m % 16, :], ps[:],
                                 AF.Gelu_apprx_sigmoid,
                                 bias=c["b1"][:, m:m + 1], scale=1.0)
        # y tiles for this half, initialized from x1
        y_tcs = []
        for tq in range(4):
            tt = half * 4 + tq
            y_tc = yp.tile([P, D], f32, tag="y", name=f"y_tc{half}_{tq}")
            nc.sync.dma_start(out=y_tc[:], in_=x1d[tt * P:(tt + 1) * P, :])
            y_tcs.append(y_tc)
        for m in range(FC):
            ps = mmO.tile([P, 512], f32, tag="mmO", name=f"psf2_{half}_{m}")
            for g in range(4):
                wt = wp.tile([P, FC, P], f32r, tag="w", name="w2t")
                nc.sync.dma_start(out=wt[:], in_=t["w2r"][m, :, g * 8:(g + 1) * 8, :])
                for kt in range(FC):
                    ktg = g * 8 + kt
                    nc.tensor.matmul(
                        ps[:], wt[:, kt, :],
                        gs[ktg // 16][:, ktg % 16, :],
                        start=(ktg == 0), stop=(ktg == MC - 1))
            mt = nst.tile([P, 512], f32, tag="nstage", name="mt")
            nc.vector.tensor_scalar_add(out=mt[:], in0=ps[:],
                                        scalar1=c["b2"][:, m:m + 1])
            mtr = nst.tile([P, 512], f32r, tag="nstage", name="mtr")
            nc.vector.tensor_copy(out=mtr[:], in_=mt[:].bitcast(f32r))
            tp = mmO.tile([P, 512], f32r, tag="mmO", name="tpmtg")
            for tq in range(4):
                nc.tensor.transpose(tp[:, tq * P:(tq + 1) * P],
                                    mtr[:, tq * P:(tq + 1) * P], ident[:])
            for tq in range(4):
                nc.vector.tensor_add(y_tcs[tq][:, m * P:(m + 1) * P],
                                     y_tcs[tq][:, m * P:(m + 1) * P],
                                     tp[:, tq * P:(tq + 1) * P].bitcast(f32))
        for tq in range(4):
            tt = half * 4 + tq
            nc.sync.dma_start(out=yd[tt * P:(tt + 1) * P, :], in_=y_tcs[tq][:])


_NC_CACHE = {}


def _get_nc(replicas=1):
    if replicas not in _NC_CACHE:
        _NC_CACHE[replicas] = build_nc(replicas)
    return _NC_CACHE[replicas]


def _retile(w, kslices, mslices):
    """[K, M] -> [mslices, 128, kslices, 128], blk[m,p,k,c] = w[k*128+p, m*128+c]."""
    K, M = w.shape
    assert K == kslices * P and M == mslices * P
    return np.ascontiguousarray(
        w.reshape(kslices, P, mslices, P).transpose(2, 1, 0, 3))


def make_in_maps(x, attention_mask, wq, bq, wk, bk, wv, bv, wo, bo,
                 ln1_s, ln1_b, ln2_s, ln2_b, w1, b1, w2, b2):
    f = lambda a: np.asarray(a, dtype=np.float32)
    wq, wk, wv, wo, w1, w2 = f(wq), f(wk), f(wv), f(wo), f(w1), f(w2)
    bq, bk, bv, bo, b1, b2 = f(bq), f(bk), f(bv), f(bo), f(b1), f(b2)
    s1, b1n, s2, b2n = f(ln1_s), f(ln1_b), f(ln2_s), f(ln2_b)
    # Fold LN affine into the consuming projections:
    #   (n*s + b) @ W + c == n @ (s[:,None]*W) + (b @ W + c)
    wq_f, bq_f = s1[:, None] * wq, b1n @ wq + bq
    wk_f, bk_f = s1[:, None] * wk, b1n @ wk + bk
    wv_f, bv_f = s1[:, None] * wv, b1n @ wv + bv
    w1_f, b1_f = s2[:, None] * w1, b2n @ w1 + b1
    shared = {
        "wqr": _retile(wq_f, FC, FC),
        "wkr": _retile(wk_f, FC, FC),
        "wvr": _retile(wv_f, FC, FC),
        "wor": _retile(wo, FC, FC),
        "w1r": _retile(w1_f, FC, MC),
        "w2r": _retile(w2, MC, FC),
        "bq": bq_f, "bk": bk_f, "bv": bv_f, "bo": bo,
        "b1": b1_f, "b2": b2,
    }
    x = f(x)
    m = np.asarray(attention_mask, dtype=np.int32)
    return [dict(shared, x=np.ascontiguousarray(x[c]),
                 mask=np.ascontiguousarray(m[c])) for c in range(NCORES)]


def kernel(**inputs):
    from concourse.bass_utils import run_bass_kernel_spmd

    nc = _get_nc()
    in_maps = make_in_maps(**inputs)
    res = run_bass_kernel_spmd(nc, in_maps, core_ids=list(range(NCORES)))
    out = np.stack([res.results[c]["y"] for c in range(NCORES)], axis=0)
    return out.astype(np.float32)
